# revision 13
# baseline (speedup 1.0000x reference)
"""Multi-head attention block kernel for Trainium2 (8 NeuronCores).

Problem: x:(2,4,1024,512) fp32, W_qkv:(512,3072), b_qkv:(3072,),
W_out:(1024,512), b_out:(512,).  out = Attention(x) per (bt,b) item.

Sharding: pure data parallel — bt*b_sz = 8 batch items, one per core.
Each core runs the full attention block on its (1024, 512) slice:
  qkv = x @ W_qkv + b_qkv           (heads=8, hd=128; scale=1/sqrt(64))
  P   = softmax(q*scale @ k^T)
  o   = (P @ v) reshaped, then o @ W_out + b_out

On-chip plan (all matmuls bf16 with fp32 PSUM accumulation):
  - x cast to bf16 (DMA cast), x^T built with DMA-xbar transposes
  - V GEMM first (Form A: n on partitions), bias via K=1 ones-row matmul,
    heads side by side with a ones column appended per head
  - then a per-head software pipeline so ScalarE exp overlaps TensorE:
      q^T,k^T projection (Form B, hd on partitions) -> S^T = k^T^T q^T
      -> P^T = exp(0.125 S^T) on ScalarE -> attn out per query-chunk with
      rhs [v | ones] (col 128 of PSUM = softmax denominator) -> reciprocal
      normalize (queries on partitions) -> PE-transpose to (h*hd, n)
  - final = out_cat^T-slices^T @ W_out + b_out (ones-row matmul bias),
    output DMA per row-chunk overlapped with the GEMM
"""

import numpy as np

P = 128
N_CTX = 1024
DIM = 512
H = 8
HD = 128
QKV = 3072
SCALE = 0.125  # (512 // 8) ** -0.5, faithful to the reference

_cached_nc = None


def _build_nc(loop_n=1):
    from contextlib import ExitStack

    import concourse.mybir as mybir
    import concourse.tile as tile
    from concourse import bacc
    from concourse.masks import make_identity

    F32 = mybir.dt.float32
    BF16 = mybir.dt.bfloat16
    AF = mybir.ActivationFunctionType

    nc = bacc.Bacc()

    x_ext = nc.declare_dram_parameter("x", [N_CTX, DIM], F32, isOutput=False)
    wqkv_ext = nc.declare_dram_parameter("W_qkv", [DIM, QKV], F32, isOutput=False)
    bqkv_ext = nc.declare_dram_parameter("b_qkv", [QKV], F32, isOutput=False)
    wout_ext = nc.declare_dram_parameter("W_out", [N_CTX, DIM], F32, isOutput=False)
    bout_ext = nc.declare_dram_parameter("b_out", [DIM], F32, isOutput=False)
    out_ext = nc.declare_dram_parameter("out", [N_CTX, DIM], F32, isOutput=True)

    NT = N_CTX // P  # 8 row tiles
    KD = DIM // P  # 4 contraction chunks for dim=512
    VW = HD + 1  # 129: v columns per head incl. ones column

    with ExitStack() as ctx:
        tc = ctx.enter_context(tile.TileContext(nc))
        consts = ctx.enter_context(tc.tile_pool(name="consts", bufs=1))
        persist = ctx.enter_context(tc.tile_pool(name="persist", bufs=1))
        work = ctx.enter_context(tc.tile_pool(name="work", bufs=2))
        small = ctx.enter_context(tc.tile_pool(name="small", bufs=3))
        dram = ctx.enter_context(tc.tile_pool(name="dram", bufs=1, space="DRAM"))
        ps_big = ctx.enter_context(tc.tile_pool(name="ps_big", bufs=2, space="PSUM"))
        ps_bank = ctx.enter_context(tc.tile_pool(name="ps_bank", bufs=4, space="PSUM"))

        # ---- constants / weights (outside any bench loop) -------------------
        ident = consts.tile([P, P], BF16, tag="ident")
        make_identity(nc, ident)
        ones_row = consts.tile([1, P], BF16, tag="ones_row")
        nc.vector.memset(ones_row, 1.0)

        # x via HWDGE fp32 (sync queue), cast + PE-transposed on chip —
        # keeps the gpsimd (SWDGE cast) queue free for the weight loads
        x_sb = persist.tile([P, NT, DIM], F32, tag="x_sb")
        for half in range(2):
            t4 = slice(half * NT // 2, (half + 1) * NT // 2)
            nc.sync.dma_start(
                x_sb[:, t4, :], x_ext.rearrange("(t p) d -> p t d", p=P)[:, t4, :]
            )

        # W_qkv as (p, ko, 3072) bf16 — contraction dim on partitions.
        # v columns first (the V GEMM runs first), then q|k per k-chunk.
        wq_sb = consts.tile([P, KD, QKV], BF16, tag="wq")
        wq_r = wqkv_ext.rearrange("(ko p) n -> p ko n", p=P)
        for k in range(KD):
            nc.gpsimd.dma_start(wq_sb[:, k, 2 * H * P :], wq_r[:, k, 2 * H * P :])
        for part in range(2):  # 0: q cols, 1: k cols
            for k in range(KD):
                sl = slice(part * H * P, (part + 1) * H * P)
                nc.gpsimd.dma_start(wq_sb[:, k, sl], wq_r[:, k, sl])
        # W_out as (p, kh, 512) bf16 — contraction dim (h*hd) on partitions
        wout_sb = consts.tile([P, H, DIM], BF16, tag="wout")
        nc.gpsimd.dma_start(wout_sb, wout_ext.rearrange("(kh p) c -> p kh c", p=P))
        # q/k bias in partition-major layout: bqk[p, m] = b_qkv[m*128 + p]
        bqk_sb = consts.tile([P, 2 * H], F32, tag="bqk")
        nc.sync.dma_start(
            bqk_sb, bqkv_ext[0 : 2 * H * P].rearrange("(t p) -> p t", p=P)
        )
        # v bias and out bias as single-partition rows (bf16, for K=1 matmuls)
        bv_row = consts.tile([1, H * HD], BF16, tag="bv")
        nc.gpsimd.dma_start(bv_row, bqkv_ext[2 * H * P : QKV][None, :])
        bout_row = consts.tile([1, DIM], BF16, tag="bout")
        nc.gpsimd.dma_start(bout_row, bout_ext[None, :])

        def body(_iv=None):
            # ---- x^T: cast to bf16 on DVE, transpose 128x128 blocks on PE --
            x_bf = persist.tile([P, NT, DIM], BF16, tag="x_bf")
            for t in range(NT):
                nc.vector.tensor_copy(x_bf[:, t, :], x_sb[:, t, :])
            xT = persist.tile([P, KD, N_CTX], BF16, tag="xT")
            for t in range(NT):
                for c in range(KD):
                    tp = ps_bank.tile([P, P], BF16, tag="bank")
                    nc.tensor.transpose(tp, x_bf[:, t, c * P : (c + 1) * P], ident)
                    nc.vector.tensor_copy(xT[:, c, t * P : (t + 1) * P], tp)

            # ---- v first (Form A): n on partitions, heads side by side with
            # a ones column: v_sb[:, t, h*129+128] = 1.0 -> softmax sums ride
            # along in the attention matmul for free.
            v_sb = persist.tile([P, NT, H * VW], BF16, tag="v_sb")
            nc.vector.memset(
                v_sb.rearrange("p t (h w) -> p t h w", w=VW)[:, :, :, HD : HD + 1],
                1.0,
            )
            for t in range(NT):
                for half in range(2):
                    ps = ps_bank.tile([P, DIM], F32, tag="bank")
                    for k in range(KD):
                        nc.tensor.matmul(
                            ps,
                            xT[:, k, t * P : (t + 1) * P],
                            wq_sb[
                                :,
                                k,
                                2 * H * P + half * DIM : 2 * H * P + (half + 1) * DIM,
                            ],
                            start=(k == 0),
                            stop=False,
                        )
                    nc.tensor.matmul(
                        ps,
                        ones_row,
                        bv_row[:, half * DIM : (half + 1) * DIM],
                        start=False,
                        stop=True,
                    )
                    dst = v_sb[:, t, :].rearrange("p (h w) -> p h w", w=VW)[
                        :, half * 4 : (half + 1) * 4, 0:HD
                    ]
                    src = ps.rearrange("p (h w) -> p h w", w=HD)
                    nc.vector.tensor_copy(dst, src)

            # ---- per-head software pipeline --------------------------------
            # Engines execute their scheduled streams in-order, so the
            # EMISSION order is the schedule.  Interleave head h's scores
            # (whose PSUM slots recycle at ScalarE's exp pace) with head
            # h-1's attention matmuls so the PE never waits inline on exp;
            # the final GEMM interleaves with the last head's attention.
            # q^T of head h = Form B M-tile over qkv cols h*128..(h+1)*128,
            # k^T of head h = cols 1024+h*128... (hd on partitions).
            outT = persist.tile([P, H, N_CTX], BF16, tag="outT")
            out_sb = persist.tile([P, NT, DIM], F32, tag="out_sb")
            out_r = out_ext.rearrange("(t p) c -> p t c", p=P)

            def emit_qk(h):
                pair = []
                for part in range(2):  # 0: q, 1: k
                    m = part * H + h
                    qk = work.tile([P, N_CTX], BF16, tag=f"qkT{part}")
                    for half in range(2):
                        sl = slice(half * DIM, (half + 1) * DIM)
                        ps = ps_bank.tile([P, DIM], F32, tag="bank")
                        for k in range(KD):
                            nc.tensor.matmul(
                                ps,
                                wq_sb[:, k, m * P : (m + 1) * P],
                                xT[:, k, sl],
                                start=(k == 0),
                                stop=(k == KD - 1),
                            )
                        nc.vector.tensor_scalar_add(
                            qk[:, sl], ps, bqk_sb[:, m : m + 1]
                        )
                    pair.append(qk)
                return pair

            def emit_scores_j(qkT_pair, pT, j):
                qT_h, kT_h = qkT_pair
                ps = ps_big.tile([P, N_CTX], F32, tag="big")
                for half in range(2):
                    sl = slice(half * DIM, (half + 1) * DIM)
                    nc.tensor.matmul(
                        ps[:, sl],
                        kT_h[:, j * P : (j + 1) * P],
                        qT_h[:, sl],
                        start=True,
                        stop=True,
                    )
                nc.scalar.activation(pT[:, j, :], ps, AF.Exp, scale=SCALE)

            def emit_attn_ic(h, pT, ic):
                aps = ps_bank.tile([P, VW], F32, tag="bank")
                for j in range(NT):
                    nc.tensor.matmul(
                        aps[:, :VW],
                        pT[:, j, ic * P : (ic + 1) * P],
                        v_sb[:, j, h * VW : (h + 1) * VW],
                        start=(j == 0),
                        stop=(j == NT - 1),
                    )
                rc = small.tile([P, 1], F32, tag="rc")
                nc.vector.reciprocal(rc, aps[:, HD : HD + 1])
                at = small.tile([P, P], BF16, tag="at")
                nc.vector.tensor_scalar_mul(at, aps[:, 0:HD], rc)
                return at

            def emit_transp(h, ic, at):
                nc.sync.dma_start(
                    outT[:, h, ic * P : (ic + 1) * P], at, transpose=True
                )

            def emit_final_ic(ic):
                fps = ps_bank.tile([P, DIM], F32, tag="bank")
                for kh in range(H):
                    nc.tensor.matmul(
                        fps,
                        outT[:, kh, ic * P : (ic + 1) * P],
                        wout_sb[:, kh, :],
                        start=(kh == 0),
                        stop=False,
                    )
                nc.tensor.matmul(fps, ones_row, bout_row, start=False, stop=True)
                nc.vector.tensor_copy(out_sb[:, ic, :], fps)
                nc.sync.dma_start(out_r[:, ic, :], out_sb[:, ic, :])

            qk_prev = emit_qk(0)
            pT_prev = work.tile([P, NT, N_CTX], BF16, tag="pT")
            for j in range(NT):
                emit_scores_j(qk_prev, pT_prev, j)
            for h in range(1, H + 1):
                if h < H:
                    qk_cur = emit_qk(h)
                    pT_cur = work.tile([P, NT, N_CTX], BF16, tag="pT")
                at_prev = None
                for j in range(NT):
                    if h < H:
                        emit_scores_j(qk_cur, pT_cur, j)
                    # attention of the previous head fills the exp latency
                    at = emit_attn_ic(h - 1, pT_prev, j)
                    if at_prev is not None:
                        emit_transp(h - 1, j - 1, at_prev)
                    at_prev = at
                    if h == H and j >= 1:
                        emit_final_ic(j - 1)
                emit_transp(h - 1, NT - 1, at_prev)
                if h == H:
                    emit_final_ic(NT - 1)
                if h < H:
                    qk_prev, pT_prev = qk_cur, pT_cur

        if loop_n == 1:
            body()
        else:
            with tc.For_i(0, loop_n, 1) as iv:
                body(iv)

    nc.finalize()
    return nc


def _get_nc():
    global _cached_nc
    if _cached_nc is None:
        _cached_nc = _build_nc()
    return _cached_nc


def kernel(**inputs):
    from concourse.bass_utils import run_bass_kernel_spmd

    x = np.ascontiguousarray(np.asarray(inputs["x"], dtype=np.float32))
    W_qkv = np.ascontiguousarray(np.asarray(inputs["W_qkv"], dtype=np.float32))
    b_qkv = np.ascontiguousarray(np.asarray(inputs["b_qkv"], dtype=np.float32))
    W_out = np.ascontiguousarray(np.asarray(inputs["W_out"], dtype=np.float32))
    b_out = np.ascontiguousarray(np.asarray(inputs["b_out"], dtype=np.float32))

    bt, b_sz, n, dim = x.shape
    xs = x.reshape(bt * b_sz, n, dim)
    nc = _get_nc()
    in_maps = [
        {
            "x": np.ascontiguousarray(xs[c]),
            "W_qkv": W_qkv,
            "b_qkv": b_qkv,
            "W_out": W_out,
            "b_out": b_out,
        }
        for c in range(8)
    ]
    res = run_bass_kernel_spmd(nc, in_maps, core_ids=list(range(8)))
    outs = np.stack([np.asarray(res.results[c]["out"]) for c in range(8)])
    return outs.reshape(bt, b_sz, n, dim).astype(np.float32)


# revision 15
# speedup vs baseline: 1.0175x; 1.0175x over previous
"""Multi-head attention block kernel for Trainium2 (8 NeuronCores).

Problem: x:(2,4,1024,512) fp32, W_qkv:(512,3072), b_qkv:(3072,),
W_out:(1024,512), b_out:(512,).  out = Attention(x) per (bt,b) item.

Sharding: pure data parallel — bt*b_sz = 8 batch items, one per core.
Each core runs the full attention block on its (1024, 512) slice:
  qkv = x @ W_qkv + b_qkv           (heads=8, hd=128; scale=1/sqrt(64))
  P   = softmax(q*scale @ k^T)
  o   = (P @ v) reshaped, then o @ W_out + b_out

On-chip plan (all matmuls bf16 with fp32 PSUM accumulation):
  - x cast to bf16 (DMA cast), x^T built with DMA-xbar transposes
  - V GEMM first (Form A: n on partitions), bias via K=1 ones-row matmul,
    heads side by side with a ones column appended per head
  - then a per-head software pipeline so ScalarE exp overlaps TensorE:
      q^T,k^T projection (Form B, hd on partitions) -> S^T = k^T^T q^T
      -> P^T = exp(0.125 S^T) on ScalarE -> attn out per query-chunk with
      rhs [v | ones] (col 128 of PSUM = softmax denominator) -> reciprocal
      normalize (queries on partitions) -> PE-transpose to (h*hd, n)
  - final = out_cat^T-slices^T @ W_out + b_out (ones-row matmul bias),
    output DMA per row-chunk overlapped with the GEMM
"""

import numpy as np

P = 128
N_CTX = 1024
DIM = 512
H = 8
HD = 128
QKV = 3072
SCALE = 0.125  # (512 // 8) ** -0.5, faithful to the reference

_cached_nc = None


def _build_nc(loop_n=1):
    from contextlib import ExitStack

    import concourse.mybir as mybir
    import concourse.tile as tile
    from concourse import bacc
    from concourse.masks import make_identity

    F32 = mybir.dt.float32
    BF16 = mybir.dt.bfloat16
    AF = mybir.ActivationFunctionType

    nc = bacc.Bacc()

    x_ext = nc.declare_dram_parameter("x", [N_CTX, DIM], F32, isOutput=False)
    wqkv_ext = nc.declare_dram_parameter("W_qkv", [DIM, QKV], F32, isOutput=False)
    bqkv_ext = nc.declare_dram_parameter("b_qkv", [QKV], F32, isOutput=False)
    wout_ext = nc.declare_dram_parameter("W_out", [N_CTX, DIM], F32, isOutput=False)
    bout_ext = nc.declare_dram_parameter("b_out", [DIM], F32, isOutput=False)
    out_ext = nc.declare_dram_parameter("out", [N_CTX, DIM], F32, isOutput=True)

    NT = N_CTX // P  # 8 row tiles
    KD = DIM // P  # 4 contraction chunks for dim=512
    VW = HD + 1  # 129: v columns per head incl. ones column

    with ExitStack() as ctx:
        tc = ctx.enter_context(tile.TileContext(nc))
        consts = ctx.enter_context(tc.tile_pool(name="consts", bufs=1))
        persist = ctx.enter_context(tc.tile_pool(name="persist", bufs=1))
        work = ctx.enter_context(tc.tile_pool(name="work", bufs=2))
        small = ctx.enter_context(tc.tile_pool(name="small", bufs=3))
        dram = ctx.enter_context(tc.tile_pool(name="dram", bufs=1, space="DRAM"))
        ps_big = ctx.enter_context(tc.tile_pool(name="ps_big", bufs=2, space="PSUM"))
        ps_bank = ctx.enter_context(tc.tile_pool(name="ps_bank", bufs=4, space="PSUM"))

        # ---- constants / weights (outside any bench loop) -------------------
        ident = consts.tile([P, P], BF16, tag="ident")
        make_identity(nc, ident)
        ones_row = consts.tile([1, P], BF16, tag="ones_row")
        nc.vector.memset(ones_row, 1.0)

        # x via HWDGE fp32 (sync queue), cast + PE-transposed on chip —
        # keeps the gpsimd (SWDGE cast) queue free for the weight loads
        x_sb = persist.tile([P, NT, DIM], F32, tag="x_sb")
        for t in range(NT):
            nc.sync.dma_start(
                x_sb[:, t, :], x_ext.rearrange("(t p) d -> p t d", p=P)[:, t, :]
            )

        # W_qkv as (p, ko, 3072) bf16 — contraction dim on partitions.
        # v columns first (the V GEMM runs first), then q|k per k-chunk.
        wq_sb = consts.tile([P, KD, QKV], BF16, tag="wq")
        wq_r = wqkv_ext.rearrange("(ko p) n -> p ko n", p=P)
        for k in range(KD):
            nc.gpsimd.dma_start(wq_sb[:, k, 2 * H * P :], wq_r[:, k, 2 * H * P :])
        for part in range(2):  # 0: q cols, 1: k cols
            for k in range(KD):
                sl = slice(part * H * P, (part + 1) * H * P)
                nc.gpsimd.dma_start(wq_sb[:, k, sl], wq_r[:, k, sl])
        # W_out as (p, kh, 512) bf16 — contraction dim (h*hd) on partitions
        wout_sb = consts.tile([P, H, DIM], BF16, tag="wout")
        nc.gpsimd.dma_start(wout_sb, wout_ext.rearrange("(kh p) c -> p kh c", p=P))
        # q/k bias in partition-major layout: bqk[p, m] = b_qkv[m*128 + p]
        bqk_sb = consts.tile([P, 2 * H], F32, tag="bqk")
        nc.sync.dma_start(
            bqk_sb, bqkv_ext[0 : 2 * H * P].rearrange("(t p) -> p t", p=P)
        )
        # v bias and out bias as single-partition rows (bf16, for K=1 matmuls)
        bv_row = consts.tile([1, H * HD], BF16, tag="bv")
        nc.gpsimd.dma_start(bv_row, bqkv_ext[2 * H * P : QKV][None, :])
        bout_row = consts.tile([1, DIM], BF16, tag="bout")
        nc.gpsimd.dma_start(bout_row, bout_ext[None, :])

        def body(_iv=None):
            # ---- x^T: cast to bf16 on DVE, transpose 128x128 blocks on PE --
            x_bf = persist.tile([P, NT, DIM], BF16, tag="x_bf")
            for t in range(NT):
                nc.vector.tensor_copy(x_bf[:, t, :], x_sb[:, t, :])
            xT = persist.tile([P, KD, N_CTX], BF16, tag="xT")
            for t in range(NT):
                for c in range(KD):
                    tp = ps_bank.tile([P, P], BF16, tag="bank")
                    nc.tensor.transpose(tp, x_bf[:, t, c * P : (c + 1) * P], ident)
                    nc.vector.tensor_copy(xT[:, c, t * P : (t + 1) * P], tp)

            # ---- v first (Form A): n on partitions, heads side by side with
            # a ones column: v_sb[:, t, h*129+128] = 1.0 -> softmax sums ride
            # along in the attention matmul for free.
            v_sb = persist.tile([P, NT, H * VW], BF16, tag="v_sb")
            nc.vector.memset(
                v_sb.rearrange("p t (h w) -> p t h w", w=VW)[:, :, :, HD : HD + 1],
                1.0,
            )
            for t in range(NT):
                for half in range(2):
                    ps = ps_bank.tile([P, DIM], F32, tag="bank")
                    for k in range(KD):
                        nc.tensor.matmul(
                            ps,
                            xT[:, k, t * P : (t + 1) * P],
                            wq_sb[
                                :,
                                k,
                                2 * H * P + half * DIM : 2 * H * P + (half + 1) * DIM,
                            ],
                            start=(k == 0),
                            stop=False,
                        )
                    nc.tensor.matmul(
                        ps,
                        ones_row,
                        bv_row[:, half * DIM : (half + 1) * DIM],
                        start=False,
                        stop=True,
                    )
                    dst = v_sb[:, t, :].rearrange("p (h w) -> p h w", w=VW)[
                        :, half * 4 : (half + 1) * 4, 0:HD
                    ]
                    src = ps.rearrange("p (h w) -> p h w", w=HD)
                    nc.vector.tensor_copy(dst, src)

            # ---- per-head software pipeline --------------------------------
            # Engines execute their scheduled streams in-order, so the
            # EMISSION order is the schedule.  Interleave head h's scores
            # (whose PSUM slots recycle at ScalarE's exp pace) with head
            # h-1's attention matmuls so the PE never waits inline on exp;
            # the final GEMM interleaves with the last head's attention.
            # q^T of head h = Form B M-tile over qkv cols h*128..(h+1)*128,
            # k^T of head h = cols 1024+h*128... (hd on partitions).
            outT = persist.tile([P, H, N_CTX], BF16, tag="outT")
            out_sb = persist.tile([P, NT, DIM], F32, tag="out_sb")
            out_r = out_ext.rearrange("(t p) c -> p t c", p=P)

            def emit_qk(h):
                pair = []
                for part in range(2):  # 0: q, 1: k
                    m = part * H + h
                    qk = work.tile([P, N_CTX], BF16, tag=f"qkT{part}")
                    for half in range(2):
                        sl = slice(half * DIM, (half + 1) * DIM)
                        ps = ps_bank.tile([P, DIM], F32, tag="bank")
                        for k in range(KD):
                            nc.tensor.matmul(
                                ps,
                                wq_sb[:, k, m * P : (m + 1) * P],
                                xT[:, k, sl],
                                start=(k == 0),
                                stop=(k == KD - 1),
                            )
                        nc.vector.tensor_scalar_add(
                            qk[:, sl], ps, bqk_sb[:, m : m + 1]
                        )
                    pair.append(qk)
                return pair

            def emit_scores_j(qkT_pair, pT, j):
                qT_h, kT_h = qkT_pair
                ps = ps_big.tile([P, N_CTX], F32, tag="big")
                for half in range(2):
                    sl = slice(half * DIM, (half + 1) * DIM)
                    nc.tensor.matmul(
                        ps[:, sl],
                        kT_h[:, j * P : (j + 1) * P],
                        qT_h[:, sl],
                        start=True,
                        stop=True,
                    )
                nc.scalar.activation(pT[:, j, :], ps, AF.Exp, scale=SCALE)

            def emit_attn_ic(h, pT, ic):
                aps = ps_bank.tile([P, VW], F32, tag="bank")
                for j in range(NT):
                    nc.tensor.matmul(
                        aps[:, :VW],
                        pT[:, j, ic * P : (ic + 1) * P],
                        v_sb[:, j, h * VW : (h + 1) * VW],
                        start=(j == 0),
                        stop=(j == NT - 1),
                    )
                rc = small.tile([P, 1], F32, tag="rc")
                nc.vector.reciprocal(rc, aps[:, HD : HD + 1])
                at = small.tile([P, P], BF16, tag="at")
                nc.vector.tensor_scalar_mul(at, aps[:, 0:HD], rc)
                return at

            def emit_transp(h, ic, at):
                tp = ps_bank.tile([P, P], BF16, tag="bank")
                nc.tensor.transpose(tp, at, ident)
                nc.vector.tensor_copy(outT[:, h, ic * P : (ic + 1) * P], tp)

            def emit_final_ic(ic):
                fps = ps_bank.tile([P, DIM], F32, tag="bank")
                for kh in range(H):
                    nc.tensor.matmul(
                        fps,
                        outT[:, kh, ic * P : (ic + 1) * P],
                        wout_sb[:, kh, :],
                        start=(kh == 0),
                        stop=False,
                    )
                nc.tensor.matmul(fps, ones_row, bout_row, start=False, stop=True)
                nc.vector.tensor_copy(out_sb[:, ic, :], fps)
                nc.sync.dma_start(out_r[:, ic, :], out_sb[:, ic, :])

            qk_prev = emit_qk(0)
            pT_prev = work.tile([P, NT, N_CTX], BF16, tag="pT")
            for j in range(NT):
                emit_scores_j(qk_prev, pT_prev, j)
            for h in range(1, H + 1):
                if h < H:
                    qk_cur = emit_qk(h)
                    pT_cur = work.tile([P, NT, N_CTX], BF16, tag="pT")
                at_prev = None
                for j in range(NT):
                    if h < H:
                        emit_scores_j(qk_cur, pT_cur, j)
                    # attention of the previous head fills the exp latency
                    at = emit_attn_ic(h - 1, pT_prev, j)
                    if at_prev is not None:
                        emit_transp(h - 1, j - 1, at_prev)
                    at_prev = at
                    if h == H and j >= 1:
                        emit_final_ic(j - 1)
                emit_transp(h - 1, NT - 1, at_prev)
                if h == H:
                    emit_final_ic(NT - 1)
                if h < H:
                    qk_prev, pT_prev = qk_cur, pT_cur

        if loop_n == 1:
            body()
        else:
            with tc.For_i(0, loop_n, 1) as iv:
                body(iv)

    nc.finalize()
    return nc


def _get_nc():
    global _cached_nc
    if _cached_nc is None:
        _cached_nc = _build_nc()
    return _cached_nc


def kernel(**inputs):
    from concourse.bass_utils import run_bass_kernel_spmd

    x = np.ascontiguousarray(np.asarray(inputs["x"], dtype=np.float32))
    W_qkv = np.ascontiguousarray(np.asarray(inputs["W_qkv"], dtype=np.float32))
    b_qkv = np.ascontiguousarray(np.asarray(inputs["b_qkv"], dtype=np.float32))
    W_out = np.ascontiguousarray(np.asarray(inputs["W_out"], dtype=np.float32))
    b_out = np.ascontiguousarray(np.asarray(inputs["b_out"], dtype=np.float32))

    bt, b_sz, n, dim = x.shape
    xs = x.reshape(bt * b_sz, n, dim)
    nc = _get_nc()
    in_maps = [
        {
            "x": np.ascontiguousarray(xs[c]),
            "W_qkv": W_qkv,
            "b_qkv": b_qkv,
            "W_out": W_out,
            "b_out": b_out,
        }
        for c in range(8)
    ]
    res = run_bass_kernel_spmd(nc, in_maps, core_ids=list(range(8)))
    outs = np.stack([np.asarray(res.results[c]["out"]) for c in range(8)])
    return outs.reshape(bt, b_sz, n, dim).astype(np.float32)
